# revision 35
# baseline (speedup 1.0000x reference)
"""Multi-head attention (B=8, N=1024, D=768, H=12) on 8 TRN2 NeuronCores.

Sharding: data-parallel over batch B — one batch element per core, weights
replicated, no collectives.

Per-core layout (everything feature-major so no on-chip transposes):
  x^T [768, 1024] (host-transposed, bf16)
  Q/K feature-major [c, n]: lhsT = w_qkv block, rhs = x^T          -> QK_fm
  V token-major  [n, c]:    lhsT = x^T block,  rhs = w_qkv V cols  -> V_tm
  S^T[k, q] per (head, ktile): lhsT = K_fm [64,128], rhs = Q_fm [64,512]
     (the two heads of a pair run as concurrent row-tiled matmuls:
      partitions 0-63 / 64-127 -> tile_position (0,0)/(64,0))
  P^T = exp(SCALE * S^T) on ACT, bf16 out
  AV^T + softmax denominator in one matmul: lhsT = [V | ones] [128, 65]
     -> psum [65, q]: rows 0-63 = (P V)^T, row 64 = rowsum(P)
  normalize: sums row staged to DRAM, partition-broadcast back via a
     step-0 DMA, r = 1/s on DVE (reciprocal_approx_fast), one multiply
     writes straight into the pair-packed proj input tile
     (cross-partition-base write, 32-aligned bases only).
  proj: lhsT = w_proj block, rhs = A_fm -> out_fm [768, 1024] fp32 + bias
Host gathers out_fm per core and transposes back to [B, 1024, 768].

Emission is software-pipelined: per ktile of pair p's body, AV of pair
p comes first (ready work), then S^T of pair p+1 (its two heads chained
with sync=False ordering edges so consecutive matmuls alternate row
groups and overlap in the PE array), then a QK matmul group of pair
p+2 (mid-loop so its PSUM slots recycle mid-pair). One unified 3-slot
[128,1024] PSUM pool serves QK/V/S^T/proj (6 banks) + a 2-slot AV pool
(2 banks); six sacrificial allocations at each pair's end shift the
slot-reuse rotation so the next pair's first S^T tiles depend on
instantly-completing memsets instead of this pair's final exps. A dummy
16-element exp at kernel start preloads the ACT table under the DMA
window; proj m-blocks 0-2 prefill their first 5 contraction steps in
the last pair's exp slack.

Startup/teardown (this session's additions, ~35us off the original):
 - 16 dummy matmuls on memset tiles run right after the framework
   preamble so the PE HAM clock-gate opens (K=8/8, 2.4 GHz) before the
   first real matmul; the first ~20us otherwise runs at 1.2 GHz and
   re-throttles during DMA-paced stretches.
 - Input DMA issues are spread across the three DMA-capable queues
   (sync/gpsimd/scalar; one DMA_DIRECT2D issue costs ~0.7us of queue
   time, and a single descriptor streams at only ~45 GB/s, so the
   original single-queue plan capped early input bandwidth at
   ~150 GB/s). The first-needed x chunks are split into 64-partition
   descriptors for latency.
 - V-tile memsets fill only the 12 per-head ones-columns (12 elems vs
   780 per partition).
 - The last pair's norm uses a DVE rebase copy (the GpSimd copy it
   replaced measured 4.3us on the tail critical path) and a bf16
   recip row so the broadcast ones-matmul runs at bf16 rate.
 - Output tiles, and the out DRAM tensor, are bf16 (host casts back to
   fp32): halves the 3MB output drain; final evictions alternate
   ACT-first (DVE is backlogged with the last stage copies), and out
   DMA issues are spread across the three queues.
"""

import numpy as np
import ml_dtypes

import concourse.bass as bass
import concourse.tile as tile
from concourse import bacc, mybir

FP32 = mybir.dt.float32
BF16 = mybir.dt.bfloat16

B, N, D = 8, 1024, 768
H, HD = 12, 64
SCALE = float(HD) ** -0.5  # 0.125
CB = D // 128  # 6 contraction blocks of 128
PAIRS = H // 2  # 6 head pairs
KT = N // 128  # 8 key-token tiles
QB = N // 512  # 2 q blocks of 512
NCORES = 8


def build_attention(tc, outs, ins):
    from contextlib import ExitStack

    nc = tc.nc
    xT = ins["xT"]  # [768, 1024] bf16 dram
    wqkv = ins["w_qkv"]  # [768, 2304] bf16 dram
    wproj = ins["w_proj"]  # [768, 768] bf16 dram
    bproj = ins["b_proj"]  # [768] fp32 dram
    out = outs["out"]  # [768, 1024] fp32 dram

    Exp = mybir.ActivationFunctionType.Exp

    with ExitStack() as ctx:
        ec = ctx.enter_context
        sb_x = ec(tc.tile_pool(name="sb_x", bufs=CB))
        sb_wqk0 = ec(tc.tile_pool(name="sb_wqk0", bufs=CB))
        sb_wqkr = ec(tc.tile_pool(name="sb_wqkr", bufs=CB))
        sb_wv = ec(tc.tile_pool(name="sb_wv", bufs=CB))
        sb_wproj = ec(tc.tile_pool(name="sb_wproj", bufs=CB))
        sb_bias = ec(tc.tile_pool(name="sb_bias", bufs=1))
        sb_qk = ec(tc.tile_pool(name="sb_qk", bufs=6))
        sb_v = ec(tc.tile_pool(name="sb_v", bufs=KT))
        sb_pt = ec(tc.tile_pool(name="sb_pt", bufs=32))
        sb_stage = ec(tc.tile_pool(name="sb_stage", bufs=3))
        sb_sbc = ec(tc.tile_pool(name="sb_sbc", bufs=2))
        sb_rbc = ec(tc.tile_pool(name="sb_rbc", bufs=2))
        sb_rrow = ec(tc.tile_pool(name="sb_rrow", bufs=2))
        sb_attn = ec(tc.tile_pool(name="sb_attn", bufs=CB))
        sb_out = ec(tc.tile_pool(name="sb_out", bufs=3))
        ps_big = ec(tc.tile_pool(name="ps_big", bufs=3, space="PSUM"))
        ps_av = ec(tc.tile_pool(name="ps_av", bufs=2, space="PSUM"))
        dram = ec(tc.tile_pool(name="dram", bufs=1, space="DRAM"))

        # shadow of the "ps" tag's 3-slot round-robin rotation, so
        # emit_st_pair can force its two tiles into adjacent psum slots
        ps_pos = [0]

        def ps_alloc(shape, name):
            t = ps_big.tile(shape, FP32, name=name, tag="ps")
            ps_pos[0] = (ps_pos[0] + 1) % 3
            return t

        # ---------- warm-up: PE + ACT busy from the first post-preamble us ----
        # memset source tiles (vector queue head, no DMA deps), then dummy
        # matmuls so the HAM clock gate sees a busy PE and opens to K=8/8
        # (2.4 GHz) before the first real matmul; also preloads the ACT exp
        # table under the DMA window.
        warm = sb_bias.tile([128, 128], BF16, name="warm", tag="warm")
        nc.gpsimd.memset(warm, 0.0)
        wrhs = sb_bias.tile([128, 512], BF16, name="wrhs", tag="wrhs")
        nc.gpsimd.memset(wrhs, 0.0)
        ones_sb = sb_bias.tile([1, 64], BF16, name="ones", tag="ones")
        nc.vector.memset(ones_sb, 1.0)
        # 16 matmuls ≈ 6.8us at the cold clock: the HAM activity window is
        # free-running, so ~2 full windows of continuous PE-busy guarantees
        # the 2.4 GHz flip before the real (DMA-paced) prologue matmuls.
        warm_ps = ps_alloc([128, 512], "warm_ps")
        for _ in range(16):
            nc.tensor.matmul(warm_ps, lhsT=warm, rhs=wrhs, start=True, stop=True)

        fill_n = [0]

        def emit_filler(n):
            # dummy matmuls dropped into DMA-paced FIFO stretches of the
            # prologue: they fill what would be PE-idle gaps so the HAM
            # clock gate never sees a MID window of idle and re-throttles
            # (each re-throttle costs ~2-4us of half-clock matmuls)
            t = ps_alloc([128, 512], f"fill{fill_n[0]}")
            fill_n[0] += 1
            for _ in range(n):
                nc.tensor.matmul(t, lhsT=warm, rhs=wrhs, start=True, stop=True)

        # ---------- loads: issue spread over 3 DMA-capable queues ------------
        # A single queue issues one DMA_DIRECT2D per ~0.8us, so the baseline's
        # single-queue load plan capped early HBM input bandwidth at
        # ~2-3 in-flight descriptors (~150 GB/s). Spread issues across
        # sync/gpsimd/scalar (the DMA-capable queues), ordered by first use.
        x_sb = []
        wqk0_sb = []
        for c in range(CB):
            xt = sb_x.tile([128, N], BF16, name=f"x{c}", tag="x")
            x_sb.append(xt)
            wt = sb_wqk0.tile([128, 256], BF16, name=f"wqk0_{c}", tag="wqk0")
            wqk0_sb.append(wt)

        def _load_wqk0(c, eng):
            rows = wqkv[c * 128 : (c + 1) * 128, :]
            src = bass.AP(
                tensor=rows.tensor,
                offset=rows.offset,
                ap=[rows.ap[0], [D, 2], [1, 128]],
            )
            eng.dma_start(wqk0_sb[c].rearrange("p (a b) -> p a b", a=2), src)

        # The first QK chain is latency-critical and a single DMA descriptor
        # streams at only ~40-50 GB/s, so split each x qb0 tile into two
        # 64-partition descriptors and interleave with the matching wqk0
        # chunk: c0-2 on sync, c3-5 on gpsimd — all landed by ~11.5us.
        # x qb1 + wqkr go on scalar, wv rides the sync/gpsimd tails.
        # exp-table warm: dst must NOT be the warm tile (a write there would
        # add a WAR edge delaying the warm matmuls' LDWEIGHTS); emitting it
        # first also runs the ~2.7us ACT table load under the DMA window
        wexp = sb_bias.tile([1, 16], FP32, name="wexp", tag="wexp")
        nc.scalar.activation(wexp, warm[0:1, 0:16], Exp, scale=1.0)
        for c in range(CB):
            eng = nc.sync if c < 3 else nc.gpsimd
            _load_wqk0(c, eng)
            eng.dma_start(
                x_sb[c][0:64, 0:512], xT[c * 128 : c * 128 + 64, 0:512]
            )
            eng.dma_start(
                x_sb[c][64:128, 0:512], xT[c * 128 + 64 : (c + 1) * 128, 0:512]
            )
        # x qb1 on scalar (behind the ACT table load; needed from ~14us)
        for c in range(CB):
            nc.scalar.dma_start(
                x_sb[c][:, 512:1024], xT[c * 128 : (c + 1) * 128, 512:1024]
            )
        wv_sb = []
        for c in range(CB):
            wt = sb_wv.tile([128, D], BF16, name=f"wv{c}", tag="wv")
            eng = nc.sync if c < 3 else nc.gpsimd
            eng.dma_start(wt, wqkv[c * 128 : (c + 1) * 128, 2 * D : 3 * D])
            wv_sb.append(wt)
        wqkr_sb = []
        for c in range(CB):
            wt = sb_wqkr.tile([128, 1280], BF16, name=f"wqkr{c}", tag="wqkr")
            rows = wqkv[c * 128 : (c + 1) * 128, :]
            src = bass.AP(
                tensor=rows.tensor,
                offset=rows.offset + 128,
                ap=[rows.ap[0], [D, 2], [1, 640]],
            )
            nc.scalar.dma_start(wt.rearrange("p (a b) -> p a b", a=2), src)
            wqkr_sb.append(wt)
        bias_sb = sb_bias.tile([128, CB], FP32, name="bias")
        nc.sync.dma_start(bias_sb, bproj.rearrange("(a p) -> p a", p=128))
        s_dram = dram.tile([H, N], FP32, name="s_dram")
        wp_sb = []
        for c in range(CB):
            wt = sb_wproj.tile([128, D], BF16, name=f"wp{c}", tag="wp")
            nc.sync.dma_start(wt, wproj[c * 128 : (c + 1) * 128, :])
            wp_sb.append(wt)

        def wqk_slice(c, p, which):
            if p == 0:
                return wqk0_sb[c][:, which * 128 : (which + 1) * 128]
            return wqkr_sb[c][:, which * 640 + (p - 1) * 128 : which * 640 + p * 128]

        qk_sb = {}  # (which, pair) -> [128, N] bf16

        def emit_qk_group(p, which, qb):
            if (which, p) not in qk_sb:
                qkt = sb_qk.tile([128, N], BF16, name=f"qk{which}_{p}", tag="qk")
                qk_sb[(which, p)] = qkt
            qkt = qk_sb[(which, p)]
            ps = ps_alloc([128, 512], f"qkps{which}_{p}_{qb}")
            for c in range(CB):
                nc.tensor.matmul(
                    ps,
                    lhsT=wqk_slice(c, p, which),
                    rhs=x_sb[c][:, qb * 512 : (qb + 1) * 512],
                    start=(c == 0),
                    stop=(c == CB - 1),
                )
            nc.vector.tensor_copy(qkt[:, qb * 512 : (qb + 1) * 512], ps)

        def emit_qk(p):
            for qb in range(QB):
                for which in (0, 1):  # 0 = Q, 1 = K
                    emit_qk_group(p, which, qb)

        pt_tiles = {}  # (pair, j, kt) -> [128, N] bf16
        from concourse.tile import add_dep_helper

        def pt_src(halves):
            # halves are two contiguous views of one [128, N] psum tile
            full = halves[0]
            return bass.AP(
                tensor=full.tensor,
                offset=full.offset,
                ap=[full.ap[0], [1, N]],
            )

        def emit_st_pair(p, kt):
            # Both heads' S^T for this ktile with alternating row groups
            # (partitions 0-63 / 64-127) so consecutive matmuls overlap in
            # the PE array (concurrent row-tiled execution).
            q_t, k_t = qk_sb[(0, p)], qk_sb[(1, p)]
            sts = []
            for j in (0, 1):
                st = ps_alloc([128, N], f"st{2*p+j}_{kt}")
                sts.append([st[:, 0:512], st[:, 512:1024]])
            prev_mm = None
            for qb in range(QB):
                for j in (0, 1):
                    mm = nc.tensor.matmul(
                        sts[j][qb],
                        lhsT=k_t[j * 64 : (j + 1) * 64, kt * 128 : (kt + 1) * 128],
                        rhs=q_t[j * 64 : (j + 1) * 64, qb * 512 : (qb + 1) * 512],
                        start=True,
                        stop=True,
                    )
                    # sync=False ordering chain: forces strict j0/j1
                    # alternation in the static PE order so consecutive
                    # S^T matmuls land on different row groups and overlap
                    # in the array (no runtime semaphore cost)
                    if prev_mm is not None:
                        add_dep_helper(
                            mm.ins,
                            prev_mm.ins,
                            sync=False,
                            reason="alternate row groups for PE overlap",
                        )
                    prev_mm = mm
            for j in (0, 1):
                pt = sb_pt.tile([128, N], BF16, name=f"pt{2*p+j}_{kt}", tag="pt")
                nc.scalar.activation(pt, pt_src(sts[j]), Exp, scale=SCALE)
                pt_tiles[(p, j, kt)] = pt

        # ---------- prologue: QK(0), then S^T/exp(0) interleaved with V ----
        v_sb = []

        def emit_v(t):
            vt = sb_v.tile([128, H * 65], BF16, name=f"v{t}", tag="v")
            # only the 12 per-head denominator columns need the 1.0 fill;
            # the V copies below cover every other column (12 elems vs 780)
            vtr_full = vt.rearrange("p (h e) -> p h e", h=H)
            nc.vector.memset(vtr_full[:, :, HD : HD + 1], 1.0)
            vtr = vtr_full[:, :, 0:HD]
            for n0, nw in ((0, 512), (512, 256)):
                vps = ps_alloc([128, nw], f"vps{t}_{n0}")
                for c in range(CB):
                    nc.tensor.matmul(
                        vps,
                        lhsT=x_sb[c][:, t * 128 : (t + 1) * 128],
                        rhs=wv_sb[c][:, n0 : n0 + nw],
                        start=(c == 0),
                        stop=(c == CB - 1),
                    )
                # copy into the 65-strided layout: n0=0 covers heads 0-7,
                # n0=512 covers heads 8-11
                h0, h1 = n0 // HD, (n0 + nw) // HD
                nc.vector.tensor_copy(
                    vtr[:, h0:h1, :],
                    vps.rearrange("p (h e) -> p h e", e=HD),
                )
            v_sb.append(vt)

        emit_qk(0)
        emit_filler(3)
        emit_filler(3)
        for kt in range(KT):
            emit_st_pair(0, kt)
            if kt < 2:
                emit_filler(2)
            if kt >= 2:
                emit_v(kt - 2)
        for t in range(KT - 2, KT):
            emit_v(t)
        emit_qk(1)

        # ---------- pipelined pairs ----------
        def emit_av_kt(p, j, av_tiles, kt):
            h = 2 * p + j
            for qb in range(QB):
                nc.tensor.matmul(
                    av_tiles[qb],
                    lhsT=v_sb[kt][:, h * 65 : (h + 1) * 65],
                    rhs=pt_tiles[(p, j, kt)][:, qb * 512 : (qb + 1) * 512],
                    start=(kt == 0),
                    stop=(kt == KT - 1),
                )

        def emit_norm(p, j, stage, at):
            h = 2 * p + j
            nc.sync.dma_start(s_dram[h : h + 1, :], stage[64:65, :])
            sbc = sb_sbc.tile([64, N], FP32, name=f"sbc{h}", tag="sbc")
            src = s_dram[h : h + 1, :]
            bcast = bass.AP(
                tensor=src.tensor, offset=src.offset, ap=[[0, 64]] + src.ap[-1:]
            )
            nc.gpsimd.dma_start(sbc, bcast)
            rbc = sb_rbc.tile([64, N], FP32, name=f"rbc{h}", tag="rbc")
            nc.vector.reciprocal_approx_fast(rbc, sbc)
            nc.vector.tensor_mul(at[j * 64 : (j + 1) * 64, :], stage[0:64, :], rbc)

        def emit_norm_fast(p, j, stage, at):
            # Low-latency variant for the final heads (pre-proj critical
            # path): DVE rebase (the GpSimd copy here measured 4.3us!) +
            # DVE recip + K=1 bf16 ones-matmul broadcast on the PE.
            h = 2 * p + j
            srow = sb_rrow.tile([1, N], FP32, name=f"srow{h}", tag="rrow")
            nc.vector.tensor_copy(srow, stage[64:65, :])
            rrowf = sb_rrow.tile([1, N], FP32, name=f"rrowf{h}", tag="rrow")
            nc.vector.reciprocal_approx_fast(rrowf, srow)
            # bf16 copy so the broadcast ones-matmul runs at bf16 rate
            # (fp32 matmuls take 4 cycles/row)
            rrow = sb_rrow.tile([1, N], BF16, name=f"rrow{h}", tag="rrowb")
            nc.vector.tensor_copy(rrow, rrowf)
            rps = ps_av.tile([64, 512], FP32, name=f"rps{h}0", tag="av")
            rps1 = ps_av.tile([64, 512], FP32, name=f"rps{h}1", tag="av")
            for qb, rp in enumerate((rps, rps1)):
                nc.tensor.matmul(
                    rp,
                    lhsT=ones_sb,
                    rhs=rrow[:, qb * 512 : (qb + 1) * 512],
                    start=True,
                    stop=True,
                )
                nc.vector.tensor_mul(
                    at[j * 64 : (j + 1) * 64, qb * 512 : (qb + 1) * 512],
                    stage[0:64, qb * 512 : (qb + 1) * 512],
                    rp,
                )

        attn_sb = []

        proj_ps = {}

        def emit_proj_k(mb, c_lo, c_hi):
            if mb not in proj_ps:
                t = ps_alloc([128, N], f"projps{mb}")
                proj_ps[mb] = [t[:, 0:512], t[:, 512:1024]]
            for qb in range(QB):
                for c in range(c_lo, c_hi):
                    nc.tensor.matmul(
                        proj_ps[mb][qb],
                        lhsT=wp_sb[c][:, mb * 128 : (mb + 1) * 128],
                        rhs=attn_sb[c][:, qb * 512 : (qb + 1) * 512],
                        start=(c == 0),
                        stop=(c == CB - 1),
                    )

        def emit_proj_out(mb):
            # alternate bias-evictions between DVE and the (tail-idle) ACT
            # so the final k5 matmuls aren't gated on one engine's queue;
            # bf16 out halves the final 3MB output DMA drain
            ot = sb_out.tile([128, N], BF16, name=f"out{mb}", tag="out")
            for qb in range(QB):
                dst = ot[:, qb * 512 : (qb + 1) * 512]
                # ACT first: DVE is backlogged with the last pair's stage
                # copies and norm muls when mb0-2 evict
                if (mb + qb) % 2 == 0:
                    nc.scalar.add(dst, proj_ps[mb][qb], bias_sb[:, mb : mb + 1])
                else:
                    nc.vector.tensor_scalar_add(
                        dst, proj_ps[mb][qb], bias_sb[:, mb : mb + 1]
                    )
                # spread issue cost (~0.6us per DMA_DIRECT2D) over queues
                eng = (nc.sync, nc.gpsimd, nc.scalar)[(2 * mb + qb) % 3]
                eng.dma_start(
                    out[mb * 128 : (mb + 1) * 128, qb * 512 : (qb + 1) * 512],
                    dst,
                )

        for p in range(PAIRS):
            at = sb_attn.tile([128, N], BF16, name=f"attn{p}", tag="attn")
            attn_sb.append(at)

            # AV(p) head 0, interleaved with S^T/exp of pair p+1 and the
            # QK matmul groups of pair p+2 (mid-loop so their PSUM slots
            # recycle mid-pair, not at the boundary)
            stage0 = sb_stage.tile([65, N], FP32, name=f"stage{2*p}", tag="stage")
            av0 = [
                ps_av.tile([65, 512], FP32, name=f"av{2*p}_{qb}", tag="av")
                for qb in range(QB)
            ]
            if p == 0 and PAIRS > 1:
                # pipeline fill: AV(0)'s chains trail the just-started exp
                # stream and stall the strict tensor FIFO; front-load the
                # first S^T(1) quads (ready work that also keeps the ACT
                # exp stream fed) ahead of them
                for kt in range(4):
                    emit_st_pair(1, kt)
            for kt in range(KT):
                emit_av_kt(p, 0, av0, kt)
                if p + 1 < PAIRS and (p != 0 or kt >= 4):
                    emit_st_pair(p + 1, kt)
                if p + 2 < PAIRS and 2 <= kt <= 5:
                    qb_, which_ = divmod(kt - 2, 2)
                    emit_qk_group(p + 2, which_, qb_)
            if p + 1 < PAIRS:
                # sacrificial ps_big allocations: shift the slot-reuse
                # rotation so the next pair's first S^T tiles depend on
                # instantly-completing memsets instead of this pair's
                # final exps (keeps ACT gapless across the boundary)
                for s in range(6):
                    sac = ps_alloc([128, 8], f"sac{p}_{s}")
                    nc.vector.memset(sac[0:1, 0:8], 0.0)
            for qb in range(QB):
                nc.vector.tensor_copy(stage0[:, qb * 512 : (qb + 1) * 512], av0[qb])
            last = p == PAIRS - 1
            # AV(p) head 1 (allocations precede the head-0 norm so the
            # fast-norm rps tiles land after them in the ps_av rotation)
            stage1 = sb_stage.tile([65, N], FP32, name=f"stage{2*p+1}", tag="stage")
            av1 = [
                ps_av.tile([65, 512], FP32, name=f"av{2*p+1}_{qb}", tag="av")
                for qb in range(QB)
            ]
            emit_norm(p, 0, stage0, at)
            for kt in range(KT):
                emit_av_kt(p, 1, av1, kt)
            if last:
                # final stage copy is on the tail critical path: split it
                # across ACT (idle after the last exps) and DVE
                nc.scalar.add(stage1[:, 0:512], av1[0], 0.0)
                nc.vector.tensor_copy(stage1[:, 512:1024], av1[1])
                emit_norm_fast(p, 1, stage1, at)
            else:
                for qb in range(QB):
                    nc.vector.tensor_copy(
                        stage1[:, qb * 512 : (qb + 1) * 512], av1[qb]
                    )
                emit_norm(p, 1, stage1, at)



        # ---------- output projection + bias ----------
        # mb0/mb1 prefill their first 5 contraction steps while the last
        # pair's normalizations finish (emitted after AV h1 so the final
        # softmax denominator chain starts as early as possible)
        emit_proj_k(0, 0, CB - 1)
        emit_proj_k(1, 0, CB - 1)
        emit_proj_k(2, 0, CB - 1)
        for mb in (0, 1, 2):
            emit_proj_k(mb, CB - 1, CB)
            emit_proj_out(mb)
        emit_proj_k(3, 0, CB)
        emit_proj_k(4, 0, CB)
        emit_proj_out(3)
        emit_proj_k(5, 0, CB)
        emit_proj_out(4)
        emit_proj_out(5)


def build_nc():
    nc = bacc.Bacc(
        "TRN2", target_bir_lowering=False, debug=False, num_devices=NCORES
    )
    ins = {
        "xT": nc.dram_tensor("xT", [D, N], BF16, kind="ExternalInput").ap(),
        "w_qkv": nc.dram_tensor("w_qkv", [D, 3 * D], BF16, kind="ExternalInput").ap(),
        "w_proj": nc.dram_tensor("w_proj", [D, D], BF16, kind="ExternalInput").ap(),
        "b_proj": nc.dram_tensor("b_proj", [D], FP32, kind="ExternalInput").ap(),
    }
    outs = {"out": nc.dram_tensor("out", [D, N], BF16, kind="ExternalOutput").ap()}
    with tile.TileContext(nc) as tc:
        build_attention(tc, outs, ins)
    nc.compile()
    return nc


def make_in_maps(x, w_qkv, w_proj, b_proj):
    xT = np.ascontiguousarray(
        np.transpose(np.asarray(x, np.float32), (0, 2, 1))
    ).astype(ml_dtypes.bfloat16)
    wq = np.asarray(w_qkv, np.float32).astype(ml_dtypes.bfloat16)
    wp = np.asarray(w_proj, np.float32).astype(ml_dtypes.bfloat16)
    bp = np.ascontiguousarray(np.asarray(b_proj, np.float32))
    return [
        {"xT": np.ascontiguousarray(xT[b]), "w_qkv": wq, "w_proj": wp, "b_proj": bp}
        for b in range(B)
    ]


_BUILT = None


def _get_built():
    global _BUILT
    if _BUILT is None:
        _BUILT = build_nc()
    return _BUILT


def kernel(x, w_qkv, w_proj, b_proj):
    from concourse.bass_utils import run_bass_kernel_spmd

    nc = _get_built()
    in_maps = make_in_maps(x, w_qkv, w_proj, b_proj)
    res = run_bass_kernel_spmd(nc, in_maps, core_ids=list(range(NCORES)))
    return np.stack(
        [np.asarray(res.results[b]["out"]).astype(np.float32).T for b in range(B)]
    )



# revision 38
# speedup vs baseline: 1.0016x; 1.0016x over previous
"""Multi-head attention (B=8, N=1024, D=768, H=12) on 8 TRN2 NeuronCores.

Sharding: data-parallel over batch B — one batch element per core, weights
replicated, no collectives.

Per-core layout (everything feature-major so no on-chip transposes):
  x^T [768, 1024] (host-transposed, bf16)
  Q/K feature-major [c, n]: lhsT = w_qkv block, rhs = x^T          -> QK_fm
  V token-major  [n, c]:    lhsT = x^T block,  rhs = w_qkv V cols  -> V_tm
  S^T[k, q] per (head, ktile): lhsT = K_fm [64,128], rhs = Q_fm [64,512]
     (the two heads of a pair run as concurrent row-tiled matmuls:
      partitions 0-63 / 64-127 -> tile_position (0,0)/(64,0))
  P^T = exp(SCALE * S^T) on ACT, bf16 out
  AV^T + softmax denominator in one matmul: lhsT = [V | ones] [128, 65]
     -> psum [65, q]: rows 0-63 = (P V)^T, row 64 = rowsum(P)
  normalize: sums row staged to DRAM, partition-broadcast back via a
     step-0 DMA, r = 1/s on DVE (reciprocal_approx_fast), one multiply
     writes straight into the pair-packed proj input tile
     (cross-partition-base write, 32-aligned bases only).
  proj: lhsT = w_proj block, rhs = A_fm -> out_fm [768, 1024] fp32 + bias
Host gathers out_fm per core and transposes back to [B, 1024, 768].

Emission is software-pipelined: per ktile of pair p's body, AV of pair
p comes first (ready work), then S^T of pair p+1 (its two heads chained
with sync=False ordering edges so consecutive matmuls alternate row
groups and overlap in the PE array), then a QK matmul group of pair
p+2 (mid-loop so its PSUM slots recycle mid-pair). One unified 3-slot
[128,1024] PSUM pool serves QK/V/S^T/proj (6 banks) + a 2-slot AV pool
(2 banks); six sacrificial allocations at each pair's end shift the
slot-reuse rotation so the next pair's first S^T tiles depend on
instantly-completing memsets instead of this pair's final exps. A dummy
16-element exp at kernel start preloads the ACT table under the DMA
window; proj m-blocks 0-2 prefill their first 5 contraction steps in
the last pair's exp slack.

Startup/teardown (this session's additions, ~35us off the original):
 - 16 dummy matmuls on memset tiles run right after the framework
   preamble so the PE HAM clock-gate opens (K=8/8, 2.4 GHz) before the
   first real matmul; the first ~20us otherwise runs at 1.2 GHz and
   re-throttles during DMA-paced stretches.
 - Input DMA issues are spread across the three DMA-capable queues
   (sync/gpsimd/scalar; one DMA_DIRECT2D issue costs ~0.7us of queue
   time, and a single descriptor streams at only ~45 GB/s, so the
   original single-queue plan capped early input bandwidth at
   ~150 GB/s). The first-needed x chunks are split into 64-partition
   descriptors for latency.
 - V-tile memsets fill only the 12 per-head ones-columns (12 elems vs
   780 per partition).
 - The last pair's norm uses a DVE rebase copy (the GpSimd copy it
   replaced measured 4.3us on the tail critical path) and a bf16
   recip row so the broadcast ones-matmul runs at bf16 rate.
 - Output tiles, and the out DRAM tensor, are bf16 (host casts back to
   fp32): halves the 3MB output drain; final evictions alternate
   ACT-first (DVE is backlogged with the last stage copies), and out
   DMA issues are spread across the three queues.
"""

import numpy as np
import ml_dtypes

import concourse.bass as bass
import concourse.tile as tile
from concourse import bacc, mybir

FP32 = mybir.dt.float32
BF16 = mybir.dt.bfloat16

B, N, D = 8, 1024, 768
H, HD = 12, 64
SCALE = float(HD) ** -0.5  # 0.125
CB = D // 128  # 6 contraction blocks of 128
PAIRS = H // 2  # 6 head pairs
KT = N // 128  # 8 key-token tiles
QB = N // 512  # 2 q blocks of 512
NCORES = 8


def build_attention(tc, outs, ins):
    from contextlib import ExitStack

    nc = tc.nc
    xT = ins["xT"]  # [768, 1024] bf16 dram
    wqkv = ins["w_qkv"]  # [768, 2304] bf16 dram
    wproj = ins["w_proj"]  # [768, 768] bf16 dram
    bproj = ins["b_proj"]  # [768] fp32 dram
    out = outs["out"]  # [768, 1024] fp32 dram

    Exp = mybir.ActivationFunctionType.Exp

    with ExitStack() as ctx:
        ec = ctx.enter_context
        sb_x = ec(tc.tile_pool(name="sb_x", bufs=CB))
        sb_wqk0 = ec(tc.tile_pool(name="sb_wqk0", bufs=CB))
        sb_wqkr = ec(tc.tile_pool(name="sb_wqkr", bufs=CB))
        sb_wv = ec(tc.tile_pool(name="sb_wv", bufs=CB))
        sb_wproj = ec(tc.tile_pool(name="sb_wproj", bufs=CB))
        sb_bias = ec(tc.tile_pool(name="sb_bias", bufs=1))
        sb_qk = ec(tc.tile_pool(name="sb_qk", bufs=6))
        sb_v = ec(tc.tile_pool(name="sb_v", bufs=KT))
        sb_pt = ec(tc.tile_pool(name="sb_pt", bufs=32))
        sb_stage = ec(tc.tile_pool(name="sb_stage", bufs=3))
        sb_sbc = ec(tc.tile_pool(name="sb_sbc", bufs=2))
        sb_rbc = ec(tc.tile_pool(name="sb_rbc", bufs=2))
        sb_rrow = ec(tc.tile_pool(name="sb_rrow", bufs=2))
        sb_attn = ec(tc.tile_pool(name="sb_attn", bufs=CB))
        sb_out = ec(tc.tile_pool(name="sb_out", bufs=3))
        ps_big = ec(tc.tile_pool(name="ps_big", bufs=3, space="PSUM"))
        ps_av = ec(tc.tile_pool(name="ps_av", bufs=2, space="PSUM"))
        dram = ec(tc.tile_pool(name="dram", bufs=1, space="DRAM"))

        # shadow of the "ps" tag's 3-slot round-robin rotation, so
        # emit_st_pair can force its two tiles into adjacent psum slots
        ps_pos = [0]

        def ps_alloc(shape, name):
            t = ps_big.tile(shape, FP32, name=name, tag="ps")
            ps_pos[0] = (ps_pos[0] + 1) % 3
            return t

        # ---------- warm-up: PE + ACT busy from the first post-preamble us ----
        # memset source tiles (vector queue head, no DMA deps), then dummy
        # matmuls so the HAM clock gate sees a busy PE and opens to K=8/8
        # (2.4 GHz) before the first real matmul; also preloads the ACT exp
        # table under the DMA window.
        warm = sb_bias.tile([128, 128], BF16, name="warm", tag="warm")
        nc.gpsimd.memset(warm, 0.0)
        wrhs = sb_bias.tile([128, 512], BF16, name="wrhs", tag="wrhs")
        nc.gpsimd.memset(wrhs, 0.0)
        ones_sb = sb_bias.tile([1, 64], BF16, name="ones", tag="ones")
        nc.vector.memset(ones_sb, 1.0)
        # 16 matmuls ≈ 6.8us at the cold clock: the HAM activity window is
        # free-running, so ~2 full windows of continuous PE-busy guarantees
        # the 2.4 GHz flip before the real (DMA-paced) prologue matmuls.
        warm_ps = ps_alloc([128, 512], "warm_ps")
        for _ in range(16):
            nc.tensor.matmul(warm_ps, lhsT=warm, rhs=wrhs, start=True, stop=True)

        fill_n = [0]

        def emit_filler(n):
            # dummy matmuls dropped into DMA-paced FIFO stretches of the
            # prologue: they fill what would be PE-idle gaps so the HAM
            # clock gate never sees a MID window of idle and re-throttles
            # (each re-throttle costs ~2-4us of half-clock matmuls)
            t = ps_alloc([128, 512], f"fill{fill_n[0]}")
            fill_n[0] += 1
            for _ in range(n):
                nc.tensor.matmul(t, lhsT=warm, rhs=wrhs, start=True, stop=True)

        # ---------- loads: issue spread over 3 DMA-capable queues ------------
        # A single queue issues one DMA_DIRECT2D per ~0.8us, so the baseline's
        # single-queue load plan capped early HBM input bandwidth at
        # ~2-3 in-flight descriptors (~150 GB/s). Spread issues across
        # sync/gpsimd/scalar (the DMA-capable queues), ordered by first use.
        x_sb = []
        wqk0_sb = []
        for c in range(CB):
            xt = sb_x.tile([128, N], BF16, name=f"x{c}", tag="x")
            x_sb.append(xt)
            wt = sb_wqk0.tile([128, 256], BF16, name=f"wqk0_{c}", tag="wqk0")
            wqk0_sb.append(wt)

        def _load_wqk0(c, eng):
            rows = wqkv[c * 128 : (c + 1) * 128, :]
            src = bass.AP(
                tensor=rows.tensor,
                offset=rows.offset,
                ap=[rows.ap[0], [D, 2], [1, 128]],
            )
            eng.dma_start(wqk0_sb[c].rearrange("p (a b) -> p a b", a=2), src)

        # The first QK chain is latency-critical and a single DMA descriptor
        # streams at only ~40-50 GB/s, so split each x qb0 tile into two
        # 64-partition descriptors and interleave with the matching wqk0
        # chunk: c0-2 on sync, c3-5 on gpsimd — all landed by ~11.5us.
        # x qb1 + wqkr go on scalar, wv rides the sync/gpsimd tails.
        # exp-table warm: dst must NOT be the warm tile (a write there would
        # add a WAR edge delaying the warm matmuls' LDWEIGHTS); emitting it
        # first also runs the ~2.7us ACT table load under the DMA window
        wexp = sb_bias.tile([1, 16], FP32, name="wexp", tag="wexp")
        nc.scalar.activation(wexp, warm[0:1, 0:16], Exp, scale=1.0)
        for c in range(CB):
            eng = nc.sync if c < 3 else nc.gpsimd
            _load_wqk0(c, eng)
            eng.dma_start(
                x_sb[c][0:64, 0:512], xT[c * 128 : c * 128 + 64, 0:512]
            )
            eng.dma_start(
                x_sb[c][64:128, 0:512], xT[c * 128 + 64 : (c + 1) * 128, 0:512]
            )
        # x qb1 split 3 ways: c0-2 on scalar (right behind the table load),
        # c3 on sync, c4-5 on gpsimd — all landed by ~14.5us
        for c in range(3):
            nc.scalar.dma_start(
                x_sb[c][:, 512:1024], xT[c * 128 : (c + 1) * 128, 512:1024]
            )
        nc.sync.dma_start(
            x_sb[3][:, 512:1024], xT[3 * 128 : 4 * 128, 512:1024]
        )
        for c in range(4, CB):
            nc.gpsimd.dma_start(
                x_sb[c][:, 512:1024], xT[c * 128 : (c + 1) * 128, 512:1024]
            )
        wv_sb = []
        for c in range(CB):
            wt = sb_wv.tile([128, D], BF16, name=f"wv{c}", tag="wv")
            eng = nc.sync if c < 3 else nc.gpsimd
            eng.dma_start(wt, wqkv[c * 128 : (c + 1) * 128, 2 * D : 3 * D])
            wv_sb.append(wt)
        wqkr_sb = []
        for c in range(CB):
            wt = sb_wqkr.tile([128, 1280], BF16, name=f"wqkr{c}", tag="wqkr")
            rows = wqkv[c * 128 : (c + 1) * 128, :]
            src = bass.AP(
                tensor=rows.tensor,
                offset=rows.offset + 128,
                ap=[rows.ap[0], [D, 2], [1, 640]],
            )
            nc.scalar.dma_start(wt.rearrange("p (a b) -> p a b", a=2), src)
            wqkr_sb.append(wt)
        bias_sb = sb_bias.tile([128, CB], FP32, name="bias")
        nc.sync.dma_start(bias_sb, bproj.rearrange("(a p) -> p a", p=128))
        s_dram = dram.tile([H, N], FP32, name="s_dram")
        wp_sb = []
        for c in range(CB):
            wt = sb_wproj.tile([128, D], BF16, name=f"wp{c}", tag="wp")
            nc.sync.dma_start(wt, wproj[c * 128 : (c + 1) * 128, :])
            wp_sb.append(wt)

        def wqk_slice(c, p, which):
            if p == 0:
                return wqk0_sb[c][:, which * 128 : (which + 1) * 128]
            return wqkr_sb[c][:, which * 640 + (p - 1) * 128 : which * 640 + p * 128]

        qk_sb = {}  # (which, pair) -> [128, N] bf16

        def emit_qk_group(p, which, qb):
            if (which, p) not in qk_sb:
                qkt = sb_qk.tile([128, N], BF16, name=f"qk{which}_{p}", tag="qk")
                qk_sb[(which, p)] = qkt
            qkt = qk_sb[(which, p)]
            ps = ps_alloc([128, 512], f"qkps{which}_{p}_{qb}")
            for c in range(CB):
                nc.tensor.matmul(
                    ps,
                    lhsT=wqk_slice(c, p, which),
                    rhs=x_sb[c][:, qb * 512 : (qb + 1) * 512],
                    start=(c == 0),
                    stop=(c == CB - 1),
                )
                # pair-0's chains are paced by per-chunk DMA arrival
                # (~0.7us apart vs 0.2us matmuls): drop fillers into the
                # FIFO mid-chain so the HAM never sees the PE idle
                if p == 0 and c in (1, 3):
                    emit_filler(2)
            nc.vector.tensor_copy(qkt[:, qb * 512 : (qb + 1) * 512], ps)

        def emit_qk(p):
            for qb in range(QB):
                for which in (0, 1):  # 0 = Q, 1 = K
                    emit_qk_group(p, which, qb)

        pt_tiles = {}  # (pair, j, kt) -> [128, N] bf16
        from concourse.tile import add_dep_helper

        def pt_src(halves):
            # halves are two contiguous views of one [128, N] psum tile
            full = halves[0]
            return bass.AP(
                tensor=full.tensor,
                offset=full.offset,
                ap=[full.ap[0], [1, N]],
            )

        def emit_st_pair(p, kt):
            # Both heads' S^T for this ktile with alternating row groups
            # (partitions 0-63 / 64-127) so consecutive matmuls overlap in
            # the PE array (concurrent row-tiled execution).
            q_t, k_t = qk_sb[(0, p)], qk_sb[(1, p)]
            sts = []
            for j in (0, 1):
                st = ps_alloc([128, N], f"st{2*p+j}_{kt}")
                sts.append([st[:, 0:512], st[:, 512:1024]])
            prev_mm = None
            for qb in range(QB):
                for j in (0, 1):
                    mm = nc.tensor.matmul(
                        sts[j][qb],
                        lhsT=k_t[j * 64 : (j + 1) * 64, kt * 128 : (kt + 1) * 128],
                        rhs=q_t[j * 64 : (j + 1) * 64, qb * 512 : (qb + 1) * 512],
                        start=True,
                        stop=True,
                    )
                    # sync=False ordering chain: forces strict j0/j1
                    # alternation in the static PE order so consecutive
                    # S^T matmuls land on different row groups and overlap
                    # in the array (no runtime semaphore cost)
                    if prev_mm is not None:
                        add_dep_helper(
                            mm.ins,
                            prev_mm.ins,
                            sync=False,
                            reason="alternate row groups for PE overlap",
                        )
                    prev_mm = mm
            for j in (0, 1):
                pt = sb_pt.tile([128, N], BF16, name=f"pt{2*p+j}_{kt}", tag="pt")
                nc.scalar.activation(pt, pt_src(sts[j]), Exp, scale=SCALE)
                pt_tiles[(p, j, kt)] = pt

        # ---------- prologue: QK(0), then S^T/exp(0) interleaved with V ----
        v_sb = []

        def emit_v(t):
            vt = sb_v.tile([128, H * 65], BF16, name=f"v{t}", tag="v")
            # only the 12 per-head denominator columns need the 1.0 fill;
            # the V copies below cover every other column (12 elems vs 780)
            vtr_full = vt.rearrange("p (h e) -> p h e", h=H)
            nc.vector.memset(vtr_full[:, :, HD : HD + 1], 1.0)
            vtr = vtr_full[:, :, 0:HD]
            for n0, nw in ((0, 512), (512, 256)):
                vps = ps_alloc([128, nw], f"vps{t}_{n0}")
                for c in range(CB):
                    nc.tensor.matmul(
                        vps,
                        lhsT=x_sb[c][:, t * 128 : (t + 1) * 128],
                        rhs=wv_sb[c][:, n0 : n0 + nw],
                        start=(c == 0),
                        stop=(c == CB - 1),
                    )
                # copy into the 65-strided layout: n0=0 covers heads 0-7,
                # n0=512 covers heads 8-11
                h0, h1 = n0 // HD, (n0 + nw) // HD
                nc.vector.tensor_copy(
                    vtr[:, h0:h1, :],
                    vps.rearrange("p (h e) -> p h e", e=HD),
                )
            v_sb.append(vt)

        emit_qk(0)
        for kt in range(KT):
            emit_st_pair(0, kt)
            if kt < 2:
                emit_filler(2)
            if kt >= 2:
                emit_v(kt - 2)
        for t in range(KT - 2, KT):
            emit_v(t)
        emit_qk(1)

        # ---------- pipelined pairs ----------
        def emit_av_kt(p, j, av_tiles, kt):
            h = 2 * p + j
            for qb in range(QB):
                nc.tensor.matmul(
                    av_tiles[qb],
                    lhsT=v_sb[kt][:, h * 65 : (h + 1) * 65],
                    rhs=pt_tiles[(p, j, kt)][:, qb * 512 : (qb + 1) * 512],
                    start=(kt == 0),
                    stop=(kt == KT - 1),
                )

        def emit_norm(p, j, stage, at):
            h = 2 * p + j
            nc.sync.dma_start(s_dram[h : h + 1, :], stage[64:65, :])
            sbc = sb_sbc.tile([64, N], FP32, name=f"sbc{h}", tag="sbc")
            src = s_dram[h : h + 1, :]
            bcast = bass.AP(
                tensor=src.tensor, offset=src.offset, ap=[[0, 64]] + src.ap[-1:]
            )
            nc.gpsimd.dma_start(sbc, bcast)
            rbc = sb_rbc.tile([64, N], FP32, name=f"rbc{h}", tag="rbc")
            nc.vector.reciprocal_approx_fast(rbc, sbc)
            nc.vector.tensor_mul(at[j * 64 : (j + 1) * 64, :], stage[0:64, :], rbc)

        def emit_norm_fast(p, j, stage, at):
            # Low-latency variant for the final heads (pre-proj critical
            # path): DVE rebase (the GpSimd copy here measured 4.3us!) +
            # DVE recip + K=1 bf16 ones-matmul broadcast on the PE.
            h = 2 * p + j
            srow = sb_rrow.tile([1, N], FP32, name=f"srow{h}", tag="rrow")
            nc.vector.tensor_copy(srow, stage[64:65, :])
            rrowf = sb_rrow.tile([1, N], FP32, name=f"rrowf{h}", tag="rrow")
            nc.vector.reciprocal_approx_fast(rrowf, srow)
            # bf16 copy so the broadcast ones-matmul runs at bf16 rate
            # (fp32 matmuls take 4 cycles/row)
            rrow = sb_rrow.tile([1, N], BF16, name=f"rrow{h}", tag="rrowb")
            nc.vector.tensor_copy(rrow, rrowf)
            rps = ps_av.tile([64, 512], FP32, name=f"rps{h}0", tag="av")
            rps1 = ps_av.tile([64, 512], FP32, name=f"rps{h}1", tag="av")
            for qb, rp in enumerate((rps, rps1)):
                nc.tensor.matmul(
                    rp,
                    lhsT=ones_sb,
                    rhs=rrow[:, qb * 512 : (qb + 1) * 512],
                    start=True,
                    stop=True,
                )
                nc.vector.tensor_mul(
                    at[j * 64 : (j + 1) * 64, qb * 512 : (qb + 1) * 512],
                    stage[0:64, qb * 512 : (qb + 1) * 512],
                    rp,
                )

        attn_sb = []

        proj_ps = {}

        def emit_proj_k(mb, c_lo, c_hi):
            if mb not in proj_ps:
                t = ps_alloc([128, N], f"projps{mb}")
                proj_ps[mb] = [t[:, 0:512], t[:, 512:1024]]
            for qb in range(QB):
                for c in range(c_lo, c_hi):
                    nc.tensor.matmul(
                        proj_ps[mb][qb],
                        lhsT=wp_sb[c][:, mb * 128 : (mb + 1) * 128],
                        rhs=attn_sb[c][:, qb * 512 : (qb + 1) * 512],
                        start=(c == 0),
                        stop=(c == CB - 1),
                    )

        def emit_proj_out(mb):
            # alternate bias-evictions between DVE and the (tail-idle) ACT
            # so the final k5 matmuls aren't gated on one engine's queue;
            # bf16 out halves the final 3MB output DMA drain
            ot = sb_out.tile([128, N], BF16, name=f"out{mb}", tag="out")
            for qb in range(QB):
                dst = ot[:, qb * 512 : (qb + 1) * 512]
                # ACT first: DVE is backlogged with the last pair's stage
                # copies and norm muls when mb0-2 evict
                if (mb + qb) % 2 == 0:
                    nc.scalar.add(dst, proj_ps[mb][qb], bias_sb[:, mb : mb + 1])
                else:
                    nc.vector.tensor_scalar_add(
                        dst, proj_ps[mb][qb], bias_sb[:, mb : mb + 1]
                    )
                # spread issue cost (~0.6us per DMA_DIRECT2D) over queues
                eng = (nc.sync, nc.gpsimd, nc.scalar)[(2 * mb + qb) % 3]
                eng.dma_start(
                    out[mb * 128 : (mb + 1) * 128, qb * 512 : (qb + 1) * 512],
                    dst,
                )

        for p in range(PAIRS):
            at = sb_attn.tile([128, N], BF16, name=f"attn{p}", tag="attn")
            attn_sb.append(at)

            # AV(p) head 0, interleaved with S^T/exp of pair p+1 and the
            # QK matmul groups of pair p+2 (mid-loop so their PSUM slots
            # recycle mid-pair, not at the boundary)
            stage0 = sb_stage.tile([65, N], FP32, name=f"stage{2*p}", tag="stage")
            av0 = [
                ps_av.tile([65, 512], FP32, name=f"av{2*p}_{qb}", tag="av")
                for qb in range(QB)
            ]
            if p == 0 and PAIRS > 1:
                # pipeline fill: AV(0)'s chains trail the just-started exp
                # stream and stall the strict tensor FIFO; front-load the
                # first S^T(1) quads (ready work that also keeps the ACT
                # exp stream fed) ahead of them
                for kt in range(4):
                    emit_st_pair(1, kt)
            for kt in range(KT):
                emit_av_kt(p, 0, av0, kt)
                if p + 1 < PAIRS and (p != 0 or kt >= 4):
                    emit_st_pair(p + 1, kt)
                if p + 2 < PAIRS and 2 <= kt <= 5:
                    qb_, which_ = divmod(kt - 2, 2)
                    emit_qk_group(p + 2, which_, qb_)
            if p + 1 < PAIRS:
                # sacrificial ps_big allocations: shift the slot-reuse
                # rotation so the next pair's first S^T tiles depend on
                # instantly-completing memsets instead of this pair's
                # final exps (keeps ACT gapless across the boundary)
                for s in range(6):
                    sac = ps_alloc([128, 8], f"sac{p}_{s}")
                    nc.vector.memset(sac[0:1, 0:8], 0.0)
            for qb in range(QB):
                nc.vector.tensor_copy(stage0[:, qb * 512 : (qb + 1) * 512], av0[qb])
            last = p == PAIRS - 1
            # AV(p) head 1 (allocations precede the head-0 norm so the
            # fast-norm rps tiles land after them in the ps_av rotation)
            stage1 = sb_stage.tile([65, N], FP32, name=f"stage{2*p+1}", tag="stage")
            av1 = [
                ps_av.tile([65, 512], FP32, name=f"av{2*p+1}_{qb}", tag="av")
                for qb in range(QB)
            ]
            emit_norm(p, 0, stage0, at)
            for kt in range(KT):
                emit_av_kt(p, 1, av1, kt)
            if last:
                # final stage copy is on the tail critical path: split it
                # across ACT (idle after the last exps) and DVE
                nc.scalar.add(stage1[:, 0:512], av1[0], 0.0)
                nc.vector.tensor_copy(stage1[:, 512:1024], av1[1])
                emit_norm_fast(p, 1, stage1, at)
            else:
                for qb in range(QB):
                    nc.vector.tensor_copy(
                        stage1[:, qb * 512 : (qb + 1) * 512], av1[qb]
                    )
                emit_norm(p, 1, stage1, at)



        # ---------- output projection + bias ----------
        # mb0/mb1 prefill their first 5 contraction steps while the last
        # pair's normalizations finish (emitted after AV h1 so the final
        # softmax denominator chain starts as early as possible)
        emit_proj_k(0, 0, CB - 1)
        emit_proj_k(1, 0, CB - 1)
        emit_proj_k(2, 0, CB - 1)
        for mb in (0, 1, 2):
            emit_proj_k(mb, CB - 1, CB)
            emit_proj_out(mb)
        emit_proj_k(3, 0, CB)
        emit_proj_k(4, 0, CB)
        emit_proj_out(3)
        emit_proj_k(5, 0, CB)
        emit_proj_out(4)
        emit_proj_out(5)


def build_nc():
    nc = bacc.Bacc(
        "TRN2", target_bir_lowering=False, debug=False, num_devices=NCORES
    )
    ins = {
        "xT": nc.dram_tensor("xT", [D, N], BF16, kind="ExternalInput").ap(),
        "w_qkv": nc.dram_tensor("w_qkv", [D, 3 * D], BF16, kind="ExternalInput").ap(),
        "w_proj": nc.dram_tensor("w_proj", [D, D], BF16, kind="ExternalInput").ap(),
        "b_proj": nc.dram_tensor("b_proj", [D], FP32, kind="ExternalInput").ap(),
    }
    outs = {"out": nc.dram_tensor("out", [D, N], BF16, kind="ExternalOutput").ap()}
    with tile.TileContext(nc) as tc:
        build_attention(tc, outs, ins)
    nc.compile()
    return nc


def make_in_maps(x, w_qkv, w_proj, b_proj):
    xT = np.ascontiguousarray(
        np.transpose(np.asarray(x, np.float32), (0, 2, 1))
    ).astype(ml_dtypes.bfloat16)
    wq = np.asarray(w_qkv, np.float32).astype(ml_dtypes.bfloat16)
    wp = np.asarray(w_proj, np.float32).astype(ml_dtypes.bfloat16)
    bp = np.ascontiguousarray(np.asarray(b_proj, np.float32))
    return [
        {"xT": np.ascontiguousarray(xT[b]), "w_qkv": wq, "w_proj": wp, "b_proj": bp}
        for b in range(B)
    ]


_BUILT = None


def _get_built():
    global _BUILT
    if _BUILT is None:
        _BUILT = build_nc()
    return _BUILT


def kernel(x, w_qkv, w_proj, b_proj):
    from concourse.bass_utils import run_bass_kernel_spmd

    nc = _get_built()
    in_maps = make_in_maps(x, w_qkv, w_proj, b_proj)
    res = run_bass_kernel_spmd(nc, in_maps, core_ids=list(range(NCORES)))
    return np.stack(
        [np.asarray(res.results[b]["out"]).astype(np.float32).T for b in range(B)]
    )

